# revision 21
# baseline (speedup 1.0000x reference)
"""CoAttention v3 for Trainium2 (8 NeuronCores, batch data-parallel).

Reference per sample (B=64, C=1024, H=W=16, N=256, CK=128):
    kx = wk1 @ xf + bk1; ky = wk2 @ yf + bk2          [CK, N]
    vx = wv1 @ xf + bv1; vy = wv2 @ yf + bv2          [C, N]
    E  = kx^T @ ky                                     [N, N]
    energy_y = E^T  (exactly)
    attn_x = softmax_rows(E); attn_y = softmax_rows(E^T)
    ox = vx @ attn_x^T ; oy = vy @ attn_y^T
    out = gamma * o + input

Device formulation (outputs computed transposed, oT[q, c]):
    oxT[q,c] = (1/Zx(q)) * sum_p expE_T[p,q] * vxT[p,c]
    oyT[q,c] = (1/Zy(q)) * sum_p expE  [p,q] * vyT[p,c]
  where expE = exp(E) (raw, unnormalized), expE_T = exp(E^T) computed
  directly as a second small matmul, Zx/Zy harvested via the activation
  accum_out.  The softmax denominator, gamma and the fp8 descale fold
  into one per-partition scalar, applied by a single fused DVE
  scalar_tensor_tensor per output chunk:
      osb = (psum * scale) + residual        (one pass, no tmp/adds)

All projections run in fp8(e4m3) with DoubleRow perf mode (FD=512).
v3 changes vs v2: 8 warmup matmuls instead of 96 (the residual DMAs no
longer race the first pair's input DMA -- they are issued behind the
first K copy-out), fused scale+residual output path, 256KB output
store chunks issued right after each fused op, and pair-2/3
V-projections split per-sample and interleaved late to fill PE gaps.
"""

import numpy as np
from contextlib import ExitStack

import ml_dtypes

B = 64
C = 1024
N = 256
CK = 128
NCORES = 8
S = B // NCORES   # samples per core
NPAIR = S // 2    # sample pairs per core
T = C // 128      # c-chunks
P = 128

SX = 16.0      # activation fp8 scale
SW = 4096.0    # weight fp8 scale
SV = 64.0      # v-projection fp8 scale

WARMUP_MMS = 24

_BF16 = ml_dtypes.bfloat16
_FP8 = ml_dtypes.float8_e4m3


def _build_program():
    import concourse.bass as bass  # noqa: F401
    import concourse.bacc as bacc
    import concourse.tile as tile
    from concourse import mybir

    dt = mybir.dt
    AF = mybir.ActivationFunctionType
    OP = mybir.AluOpType
    DR = mybir.MatmulPerfMode.DoubleRow

    nc = bacc.Bacc()

    xb = nc.declare_dram_parameter("xb", [NPAIR, P, T, 2 * N], dt.float8e4, isOutput=False)
    yb = nc.declare_dram_parameter("yb", [NPAIR, P, T, 2 * N], dt.float8e4, isOutput=False)
    rr = nc.declare_dram_parameter("rr", [S, P, 2, 2 * C], dt.bfloat16, isOutput=False)
    wkt = nc.declare_dram_parameter("wkt", [P, 2, T, CK], dt.float8e4, isOutput=False)
    wv1t = nc.declare_dram_parameter("wv1t", [2, P, T, 512], dt.float8e4, isOutput=False)
    wv2t = nc.declare_dram_parameter("wv2t", [2, P, T, 512], dt.float8e4, isOutput=False)
    cvec = nc.declare_dram_parameter("cvec", [P, 4], dt.float32, isOutput=False)
    oo = nc.declare_dram_parameter("oo", [S, P, 2, 2 * C], dt.bfloat16, isOutput=True)

    with tile.TileContext(nc) as tc, ExitStack() as ctx:
        singles = ctx.enter_context(tc.tile_pool(name="singles", bufs=1))
        p_act = ctx.enter_context(tc.tile_pool(name="p_act", bufs=2))
        p_k = ctx.enter_context(tc.tile_pool(name="p_k", bufs=2))
        p_vt = ctx.enter_context(tc.tile_pool(name="p_vt", bufs=2))
        p_e = ctx.enter_context(tc.tile_pool(name="p_e", bufs=4))
        p_z = ctx.enter_context(tc.tile_pool(name="p_z", bufs=4))
        p_res = ctx.enter_context(tc.tile_pool(name="p_res", bufs=1))
        p_tmp = ctx.enter_context(tc.tile_pool(name="p_tmp", bufs=3))
        p_out = ctx.enter_context(tc.tile_pool(name="p_out", bufs=3))
        # PSUM: 2 + 2 + 4 = 8 banks
        ps_s = ctx.enter_context(tc.tile_pool(name="ps_s", bufs=2, space="PSUM"))
        ps_v = ctx.enter_context(tc.tile_pool(name="ps_v", bufs=2, space="PSUM"))
        ps_o = ctx.enter_context(tc.tile_pool(name="ps_o", bufs=2, space="PSUM"))

        wk_sb2 = singles.tile([P, 2, T, CK], dt.float8e4, name="wk_sb2", tag="wk_sb2")
        wk_sb = [wk_sb2[:, 0], wk_sb2[:, 1]]
        wv_sb = [[singles.tile([P, T, 512], dt.float8e4, name=f"wv{i}{h}", tag=f"wv{i}{h}")
                  for h in range(2)] for i in range(2)]
        cv_sb = singles.tile([P, 4], dt.float32, name="cv_sb", tag="cv_sb")
        bk_sb = [cv_sb[:, 0:1], cv_sb[:, 1:2]]
        g_sb = [cv_sb[:, 2:3], cv_sb[:, 3:4]]

        # short HAM warmup: keep the PE clock spinning through the DMA-bound
        # startup; the first real matmuls follow as soon as pair-0 lands
        wtile = singles.tile([P, P], dt.bfloat16, name="wtile", tag="wtile")
        nc.vector.memset(wtile[:], 0)
        for wi in range(WARMUP_MMS):
            wps = ps_s.tile([P, P], dt.float32, name="pss", tag="pss")
            nc.tensor.matmul(wps[:], wtile[:], wtile[:], start=True, stop=True)

        pre_acts = {}
        for pr in range(2):
            pre_acts[pr] = [
                p_act.tile([P, T, 2 * N], dt.float8e4, name=f"act{bi}", tag=f"act{bi}")
                for bi in range(2)
            ]
        # DMA priority: pair-0 acts + K/const weights on the sync queue, V
        # weights in parallel on the scalar queue (fewest possible triggers;
        # trigger dispatch serializes per-sequencer at ~1us each).  Pair-1
        # acts are deferred to the vector queue mid-stream, and the 8.4MB
        # residual load rides in two waves behind the K copy-outs.
        nc.sync.dma_start(out=pre_acts[0][0][:], in_=xb[0])
        nc.sync.dma_start(out=wk_sb2[:], in_=wkt[:])
        nc.sync.dma_start(out=cv_sb[:], in_=cvec[:])
        nc.sync.dma_start(out=pre_acts[0][1][:], in_=yb[0])
        for hc in range(2):
            for bi, drm in ((0, wv1t), (1, wv2t)):
                nc.scalar.dma_start(out=wv_sb[bi][hc][:], in_=drm[hc])
        res_tiles = [
            p_res.tile([P, 2, 2 * C], dt.bfloat16, name=f"res{s}", tag=f"res{s}")
            for s in range(S)
        ]

        k_state = {}
        vts_state = {}
        samp_state = {}

        def prefetch_acts(pr, eng):
            """Allocate + DMA a pair's act tiles at an explicit pipeline
            position (the trigger waits for the pool buffer, so place it on
            a queue where blocking is harmless)."""
            act = []
            for bi, drm in ((0, xb), (1, yb)):
                tb = p_act.tile([P, T, 2 * N], dt.float8e4, name=f"act{bi}", tag=f"act{bi}")
                eng.dma_start(out=tb[:], in_=drm[pr])
                act.append(tb)
            pre_acts[pr] = act

        def stage_proj_k(pr):
            """K-projection (2 samples wide)."""
            act = pre_acts.pop(pr)

            k_sb = []
            for bi in range(2):
                kps = ps_s.tile([P, 2 * N], dt.float32, name="pss", tag="pss")
                for t in range(0, T, 2):
                    nc.tensor.matmul(
                        kps[:], wk_sb[bi][:, t:t + 2, :], act[bi][:, t:t + 2, :],
                        start=(t == 0), stop=(t == T - 2), perf_mode=DR,
                    )
                ksb = p_k.tile([P, 2 * N], dt.bfloat16, name=f"k{bi}", tag=f"k{bi}")
                nc.scalar.activation(
                    ksb[:], kps[:], AF.Identity,
                    bias=bk_sb[bi][:, 0:1], scale=1.0 / (SX * SW),
                )
                k_sb.append(ksb)
            k_state[pr] = (act, k_sb)

            if pr == 0:
                # pair-1 acts + first residuals ride the scalar queue behind
                # the first K copy-out: they start once pair-0 + wv are in
                # flight, keeping the rings clear for the critical prefix.
                for bi, drm in ((0, xb), (1, yb)):
                    nc.scalar.dma_start(out=pre_acts[1][bi][:], in_=drm[1])
                for s in range(2):
                    nc.scalar.dma_start(out=res_tiles[s][:], in_=rr[s])
            elif pr == 1:
                for s in range(2, S):
                    nc.scalar.dma_start(out=res_tiles[s][:], in_=rr[s])

        def stage_proj_v(pr, si, bis=(0, 1)):
            """DoubleRow V-projections for sample si of the pair."""
            act, _ = k_state[pr]
            for bi in bis:
                vts_state[(pr, si, bi)] = p_vt.tile(
                    [P, 2, C], dt.float8e4, name=f"vt{si}{bi}", tag=f"vt{si}{bi}")
            for h in range(2):
                for bi in bis:
                    for pc in range(2):
                        vps = ps_v.tile([P, 512], dt.float32, name="psv", tag="psv")
                        for t in range(0, T, 2):
                            nc.tensor.matmul(
                                vps[:],
                                act[bi][:, t:t + 2, si * N + pc * P: si * N + (pc + 1) * P],
                                wv_sb[bi][h][:, t:t + 2, :],
                                start=(t == 0), stop=(t == T - 2), perf_mode=DR,
                            )
                        dst = vts_state[(pr, si, bi)][:, pc, h * 512:(h + 1) * 512]
                        if (pc + bi) % 2 == 0:
                            nc.scalar.mul(dst, vps[:], SV / (SX * SW))
                        else:
                            nc.vector.tensor_scalar_mul(dst, vps[:], SV / (SX * SW))

        def stage_attn(s):
            """Energy, exp (+Zx), E^T energy (+Zy), softmax scale vectors."""
            pr, si = s // 2, s % 2
            _, k_sb = k_state[pr]
            kx, ky = k_sb[0], k_sb[1]

            expE = p_e.tile([P, 2, N], dt.float8e4, name="expE", tag="expE")
            z = p_z.tile([P, 4], dt.float32, name="z", tag="z")
            zx, zy = z[:, 0:2], z[:, 2:4]
            for qa in range(2):
                eps = ps_s.tile([P, N], dt.float32, name="pss", tag="pss")
                nc.tensor.matmul(
                    eps[:],
                    kx[:, si * N + qa * P: si * N + (qa + 1) * P],
                    ky[:, si * N:(si + 1) * N],
                    start=True, stop=True,
                )
                nc.scalar.activation(
                    expE[:, qa, :], eps[:], AF.Exp, accum_out=zx[:, qa:qa + 1],
                )

            # E^T computed directly: E^T = ky^T @ kx (cheaper than PE transposes)
            expT = p_e.tile([P, 2, N], dt.float8e4, name="expT", tag="expT")
            for mi in range(2):
                etps = ps_s.tile([P, N], dt.float32, name="pss", tag="pss")
                nc.tensor.matmul(
                    etps[:],
                    ky[:, si * N + mi * P: si * N + (mi + 1) * P],
                    kx[:, si * N:(si + 1) * N],
                    start=True, stop=True,
                )
                nc.scalar.activation(
                    expT[:, mi, :], etps[:], AF.Exp, accum_out=zy[:, mi:mi + 1],
                )

            rz = p_z.tile([P, 4], dt.float32, name="rz", tag="rz")
            nc.vector.reciprocal(rz[:], z[:])
            gr = p_z.tile([P, 4], dt.float32, name="gr", tag="gr")
            for bi in range(2):
                nc.vector.tensor_scalar(
                    gr[:, 2 * bi:2 * bi + 2], rz[:, 2 * bi:2 * bi + 2],
                    g_sb[bi][:, 0:1], 1.0 / SV,
                    op0=OP.mult, op1=OP.mult,
                )

            samp_state[s] = (expE, expT, gr)

        def stage_out(s):
            """DoubleRow output matmuls + fused scale+residual + chunk store."""
            pr, si = s // 2, s % 2
            expE, expT, grg = samp_state.pop(s)
            res = res_tiles[s]

            osb = p_out.tile([P, 2, 2 * C], dt.bfloat16, name="osb", tag="osb")
            for bi in range(2):
                stat = expT if bi == 0 else expE
                vt = vts_state.pop((pr, si, bi))
                for qs in range(2):
                    ops = ps_o.tile([P, 2 * 512], dt.float32, name="pso", tag="pso")
                    sc = grg[:, 2 * bi + qs:2 * bi + qs + 1]
                    tmp = p_tmp.tile([P, C], dt.bfloat16, name="tmp", tag="tmp")
                    for h in range(2):
                        mm = nc.tensor.matmul(
                            ops[:, h * 512:(h + 1) * 512],
                            stat[:, :, qs * P:(qs + 1) * P],
                            vt[:, :, h * 512:(h + 1) * 512],
                            start=True, stop=True, perf_mode=DR,
                        )
                        if h == 1:
                            # same stationary operand as h==0: skip the reload
                            mm.ins.ldweights = False
                        # drain each 512-half as soon as its matmul stops:
                        # ACT takes h0 while DVE takes h1 (different PSUM
                        # banks), halving the PSUM hold time
                        psh = ops[:, h * 512:(h + 1) * 512]
                        tmph = tmp[:, h * 512:(h + 1) * 512]
                        if h == 0:
                            nc.scalar.activation(tmph, psh, AF.Identity, scale=sc)
                        else:
                            nc.vector.tensor_scalar(tmph, psh, sc, None, op0=OP.mult)
                        if s == S - 1:
                            dsth = osb[:, bi, qs * C + h * 512:qs * C + (h + 1) * 512]
                            rsh = res[:, bi, qs * C + h * 512:qs * C + (h + 1) * 512]
                            nc.vector.tensor_add(dsth, tmph, rsh)
                            nc.sync.dma_start(
                                out=oo[s][:, bi, qs * C + h * 512:qs * C + (h + 1) * 512],
                                in_=dsth)
                    if s == S - 1:
                        continue
                    dst = osb[:, bi, qs * C:(qs + 1) * C]
                    eng = nc.gpsimd if (bi == 1 and 1 <= s <= 4) else nc.vector
                    eng.tensor_add(dst, tmp[:], res[:, bi, qs * C:(qs + 1) * C])
                    nc.sync.dma_start(out=oo[s][:, bi, qs * C:(qs + 1) * C], in_=dst)

        # software pipeline over sample pairs; pair-2/3 V-projections are
        # split per-sample and attn stages hoisted so the final out stages
        # run dense with exps precomputed
        stage_proj_k(0)
        stage_proj_v(0, 0)
        stage_proj_v(0, 1)
        stage_proj_k(1)
        stage_proj_v(1, 0)
        stage_proj_v(1, 1)
        stage_attn(0)
        stage_attn(1)
        prefetch_acts(2, nc.sync)
        stage_out(0)
        stage_attn(2)
        stage_out(1)
        prefetch_acts(3, nc.sync)
        stage_proj_k(2)
        stage_proj_v(2, 0)
        stage_attn(3)
        stage_out(2)
        stage_attn(4)
        stage_attn(5)
        stage_proj_v(2, 1)
        stage_proj_k(3)
        stage_proj_v(3, 0)
        stage_out(3)
        stage_attn(6)
        stage_attn(7)
        stage_out(4)
        stage_proj_v(3, 1, bis=(0,))
        stage_out(5)
        stage_proj_v(3, 1, bis=(1,))
        stage_out(6)
        stage_out(7)

    nc.finalize()
    return nc


def _ensure_axon_hooks_importable():
    try:
        import antenv.axon_hooks  # noqa: F401
    except Exception:
        import sys
        import types
        m = types.ModuleType("antenv.axon_hooks")
        m.get_axon_ntff_profile_hook = lambda: None
        m.set_axon_ntff_profile_hook = lambda h: None
        sys.modules["antenv.axon_hooks"] = m


def kernel(x, y, wk1, bk1, wk2, bk2, wv1, bv1, wv2, bv2, gamma1, gamma2):
    from concourse.bass_utils import run_bass_kernel_spmd

    _ensure_axon_hooks_importable()

    x = np.asarray(x, np.float32)
    y = np.asarray(y, np.float32)
    g1v = np.float32(np.asarray(gamma1).reshape(-1)[0])
    g2v = np.float32(np.asarray(gamma2).reshape(-1)[0])

    # activations: [B,C,H,W] -> [NCORES, NPAIR, P, T, 2N] fp8 (x16)
    def act_prep(a):
        r = a.reshape(NCORES, NPAIR, 2, T, P, N).transpose(0, 1, 4, 3, 2, 5)
        return np.ascontiguousarray(r).reshape(NCORES, NPAIR, P, T, 2 * N) * np.float32(SX)

    xq = act_prep(x).astype(_FP8)
    yq = act_prep(y).astype(_FP8)

    # residuals: xT + gamma*bv  -> [NCORES, S, P, 2C] bf16
    def res_prep(a, bv, gv):
        r = a.reshape(B, C, N).transpose(0, 2, 1) + (gv * np.asarray(bv, np.float32))[None, None, :]
        r = r.reshape(B, 2, P, C).transpose(0, 2, 1, 3)
        return np.ascontiguousarray(r).reshape(NCORES, S, P, 2 * C).astype(_BF16)

    rrq = np.stack([res_prep(x, bv1, g1v), res_prep(y, bv2, g2v)], axis=3)

    def wprep(w, cols):  # [cols, C] -> [P, T, cols] fp8 (x SW)
        r = np.asarray(w, np.float32).T.reshape(T, P, cols).transpose(1, 0, 2)
        return np.ascontiguousarray(r * np.float32(SW)).astype(_FP8)

    def wvprep(w):  # [C, C] -> [2, P, T, 512] fp8 (x SW), contiguous col-halves
        r = wprep(w, C).reshape(P, T, 2, 512).transpose(2, 0, 1, 3)
        return np.ascontiguousarray(r)

    common = {
        "wkt": np.stack([wprep(wk1, CK), wprep(wk2, CK)], axis=1),
        "wv1t": wvprep(wv1), "wv2t": wvprep(wv2),
        "cvec": np.ascontiguousarray(np.stack([
            np.asarray(bk1, np.float32).reshape(P),
            np.asarray(bk2, np.float32).reshape(P),
            np.full(P, g1v, np.float32),
            np.full(P, g2v, np.float32)], axis=1)),
    }

    nc = _build_program()
    in_maps = []
    for c in range(NCORES):
        in_maps.append({
            "xb": xq[c], "yb": yq[c], "rr": rrq[c],
            **common,
        })

    global LAST_RESULTS
    LAST_RESULTS = run_bass_kernel_spmd(nc, in_maps, list(range(NCORES)))
    res = LAST_RESULTS.results

    o = np.stack([res[c]["oo"] for c in range(NCORES)])  # [NC, S, P, 2, 2C]

    def unpack(r):  # [NC, S, P, 2C] bf16 -> [B, C, H, W] f32
        r = np.asarray(r, np.float32).reshape(B, P, 2, C).transpose(0, 3, 2, 1)
        return np.ascontiguousarray(r).reshape(B, C, 16, 16)

    return unpack(o[:, :, :, 0, :]), unpack(o[:, :, :, 1, :])


# revision 22
# speedup vs baseline: 1.0454x; 1.0454x over previous
"""CoAttention v2 for Trainium2 (8 NeuronCores, batch data-parallel).

Reference per sample (B=64, C=1024, H=W=16, N=256, CK=128):
    kx = wk1 @ xf + bk1; ky = wk2 @ yf + bk2          [CK, N]
    vx = wv1 @ xf + bv1; vy = wv2 @ yf + bv2          [C, N]
    E  = kx^T @ ky                                     [N, N]
    energy_y = E^T  (exactly)
    attn_x = softmax_rows(E); attn_y = softmax_rows(E^T)
    ox = vx @ attn_x^T ; oy = vy @ attn_y^T
    out = gamma * o + input

Device formulation (outputs computed transposed, oT[q, c]):
    oxT[q,c] = (1/Zx(q)) * sum_p expE_T[p,q] * vxT[p,c]
    oyT[q,c] = (1/Zy(q)) * sum_p expE  [p,q] * vyT[p,c]
  where expE = exp(E) (raw, unnormalized), expE_T its transpose,
  Zx = row-sums of expE (free-axis accum of the exp), Zy = col-sums of
  expE == free-axis accum of expE_T (harvested during the transpose
  copy-out).  The 1/Z softmax denominators and the fp8 descale fold into
  the per-partition `scale=` of the PSUM->SBUF activation, so softmax
  costs no standalone normalize pass, and only 4 PE transposes/sample.

All projections run in fp8(e4m3) with DoubleRow perf mode (2 c-chunks
contracted per pass, FD=512): K-projection batches 2 samples to reach
FD=512; V-projection streams wv columns; output matmuls contract the
full 256 positions in a single DoubleRow matmul.  Weights are scaled by
4096 and activations by 16 on the host to clear the e4m3 subnormal
range; descales ride existing activation `scale=` constants.

The residual (x + gamma*bv, broadcast) is pre-combined on the host and
added on GPSIMD; outputs return as bf16.
"""

import numpy as np
from contextlib import ExitStack

import ml_dtypes

B = 64
C = 1024
N = 256
CK = 128
NCORES = 8
S = B // NCORES   # samples per core
NPAIR = S // 2    # sample pairs per core
T = C // 128      # c-chunks
P = 128

SX = 16.0      # activation fp8 scale
SW = 4096.0    # weight fp8 scale
SV = 64.0      # v-projection fp8 scale

_BF16 = ml_dtypes.bfloat16
_FP8 = ml_dtypes.float8_e4m3


def _build_program():
    import concourse.bass as bass  # noqa: F401
    import concourse.bacc as bacc
    import concourse.tile as tile
    from concourse import mybir

    dt = mybir.dt
    AF = mybir.ActivationFunctionType
    OP = mybir.AluOpType
    DR = mybir.MatmulPerfMode.DoubleRow

    nc = bacc.Bacc()

    xb = nc.declare_dram_parameter("xb", [NPAIR, P, T, 2 * N], dt.float8e4, isOutput=False)
    yb = nc.declare_dram_parameter("yb", [NPAIR, P, T, 2 * N], dt.float8e4, isOutput=False)
    rr = nc.declare_dram_parameter("rr", [S, P, 2, 2 * C], dt.bfloat16, isOutput=False)
    wkt = nc.declare_dram_parameter("wkt", [P, 2, T, CK], dt.float8e4, isOutput=False)
    wv1t = nc.declare_dram_parameter("wv1t", [2, P, T, 512], dt.float8e4, isOutput=False)
    wv2t = nc.declare_dram_parameter("wv2t", [2, P, T, 512], dt.float8e4, isOutput=False)
    cvec = nc.declare_dram_parameter("cvec", [P, 4], dt.float32, isOutput=False)
    oo = nc.declare_dram_parameter("oo", [S, P, 2, 2 * C], dt.bfloat16, isOutput=True)

    with tile.TileContext(nc) as tc, ExitStack() as ctx:
        singles = ctx.enter_context(tc.tile_pool(name="singles", bufs=1))
        p_act = ctx.enter_context(tc.tile_pool(name="p_act", bufs=2))
        p_k = ctx.enter_context(tc.tile_pool(name="p_k", bufs=2))
        p_vt = ctx.enter_context(tc.tile_pool(name="p_vt", bufs=2))
        p_e = ctx.enter_context(tc.tile_pool(name="p_e", bufs=3))
        p_z = ctx.enter_context(tc.tile_pool(name="p_z", bufs=4))
        p_res = ctx.enter_context(tc.tile_pool(name="p_res", bufs=1))
        p_tmp = ctx.enter_context(tc.tile_pool(name="p_tmp", bufs=2))
        p_out = ctx.enter_context(tc.tile_pool(name="p_out", bufs=2))
        # PSUM: 3 + 2 + 3 = 8 banks
        ps_s = ctx.enter_context(tc.tile_pool(name="ps_s", bufs=2, space="PSUM"))
        ps_v = ctx.enter_context(tc.tile_pool(name="ps_v", bufs=2, space="PSUM"))
        ps_o = ctx.enter_context(tc.tile_pool(name="ps_o", bufs=2, space="PSUM"))

        wk_sb2 = singles.tile([P, 2, T, CK], dt.float8e4, name="wk_sb2", tag="wk_sb2")
        wk_sb = [wk_sb2[:, 0], wk_sb2[:, 1]]
        wv_sb = [[singles.tile([P, T, 512], dt.float8e4, name=f"wv{i}{h}", tag=f"wv{i}{h}")
                  for h in range(2)] for i in range(2)]
        cv_sb = singles.tile([P, 4], dt.float32, name="cv_sb", tag="cv_sb")
        bk_sb = [cv_sb[:, 0:1], cv_sb[:, 1:2]]
        g_sb = [cv_sb[:, 2:3], cv_sb[:, 3:4]]

        # HAM warmup: keep the PE clock un-throttled through the DMA-bound
        # startup so the first real matmuls run at 2.4 GHz
        wtile = singles.tile([P, P], dt.bfloat16, name="wtile", tag="wtile")
        nc.vector.memset(wtile[:], 0)
        for wi in range(96):
            wps = ps_s.tile([P, P], dt.float32, name="pss", tag="pss")
            nc.tensor.matmul(wps[:], wtile[:], wtile[:], start=True, stop=True)

        pre_acts = {}
        for pr in range(2):
            pre_acts[pr] = [
                p_act.tile([P, T, 2 * N], dt.float8e4, name=f"act{bi}", tag=f"act{bi}")
                for bi in range(2)
            ]
        nc.sync.dma_start(out=pre_acts[0][0][:], in_=xb[0])
        nc.sync.dma_start(out=wk_sb2[:], in_=wkt[:])
        nc.sync.dma_start(out=cv_sb[:], in_=cvec[:])
        nc.sync.dma_start(out=pre_acts[0][1][:], in_=yb[0])
        nc.sync.dma_start(out=pre_acts[1][0][:], in_=xb[1])
        nc.sync.dma_start(out=pre_acts[1][1][:], in_=yb[1])
        for hc in range(2):
            for bi, drm in ((0, wv1t), (1, wv2t)):
                nc.scalar.dma_start(out=wv_sb[bi][hc][:], in_=drm[hc])
        res_tiles = []
        for s in range(S):
            rt = p_res.tile([P, 2, 2 * C], dt.bfloat16, name=f"res{s}", tag=f"res{s}")
            nc.scalar.dma_start(out=rt[:], in_=rr[s])
            res_tiles.append(rt)

        pair_state = {}
        samp_state = {}

        def stage_proj(pr):
            """DMA + K-projection (2 samples wide) + DoubleRow V-projections."""
            if pr in pre_acts:
                act = pre_acts.pop(pr)
            else:
                act = []
                for bi, drm in ((0, xb), (1, yb)):
                    tb = p_act.tile([P, T, 2 * N], dt.float8e4, name=f"act{bi}", tag=f"act{bi}")
                    nc.sync.dma_start(out=tb[:], in_=drm[pr])
                    act.append(tb)

            k_sb = []
            for bi in range(2):
                kps = ps_s.tile([P, 2 * N], dt.float32, name="pss", tag="pss")
                for t in range(0, T, 2):
                    nc.tensor.matmul(
                        kps[:], wk_sb[bi][:, t:t + 2, :], act[bi][:, t:t + 2, :],
                        start=(t == 0), stop=(t == T - 2), perf_mode=DR,
                    )
                ksb = p_k.tile([P, 2 * N], dt.bfloat16, name=f"k{bi}", tag=f"k{bi}")
                nc.scalar.activation(
                    ksb[:], kps[:], AF.Identity,
                    bias=bk_sb[bi][:, 0:1], scale=1.0 / (SX * SW),
                )
                k_sb.append(ksb)

            vts = {}
            for si in range(2):
                for bi in range(2):
                    vts[(si, bi)] = p_vt.tile(
                        [P, 2, C], dt.float8e4, name=f"vt{si}{bi}", tag=f"vt{si}{bi}")
            for h in range(2):
                for si in range(2):
                    for bi in range(2):
                        for pc in range(2):
                            vps = ps_v.tile([P, 512], dt.float32, name="psv", tag="psv")
                            for t in range(0, T, 2):
                                nc.tensor.matmul(
                                    vps[:],
                                    act[bi][:, t:t + 2, si * N + pc * P: si * N + (pc + 1) * P],
                                    wv_sb[bi][h][:, t:t + 2, :],
                                    start=(t == 0), stop=(t == T - 2), perf_mode=DR,
                                )
                            dst = vts[(si, bi)][:, pc, h * 512:(h + 1) * 512]
                            if (pc + bi) % 2 == 0:
                                nc.scalar.mul(dst, vps[:], SV / (SX * SW))
                            else:
                                nc.vector.tensor_scalar_mul(dst, vps[:], SV / (SX * SW))
            pair_state[pr] = (k_sb, vts)

        def stage_attn(s):
            """Energy, exp (+Zx), transpose (+Zy), softmax scale vectors."""
            pr, si = s // 2, s % 2
            k_sb, vts = pair_state[pr]
            kx, ky = k_sb[0], k_sb[1]

            expE = p_e.tile([P, 2, N], dt.float8e4, name="expE", tag="expE")
            z = p_z.tile([P, 4], dt.float32, name="z", tag="z")
            zx, zy = z[:, 0:2], z[:, 2:4]
            for qa in range(2):
                eps = ps_s.tile([P, N], dt.float32, name="pss", tag="pss")
                nc.tensor.matmul(
                    eps[:],
                    kx[:, si * N + qa * P: si * N + (qa + 1) * P],
                    ky[:, si * N:(si + 1) * N],
                    start=True, stop=True,
                )
                nc.scalar.activation(
                    expE[:, qa, :], eps[:], AF.Exp, accum_out=zx[:, qa:qa + 1],
                )

            # E^T computed directly: E^T = ky^T @ kx (cheaper than PE transposes)
            expT = p_e.tile([P, 2, N], dt.float8e4, name="expT", tag="expT")
            for mi in range(2):
                etps = ps_s.tile([P, N], dt.float32, name="pss", tag="pss")
                nc.tensor.matmul(
                    etps[:],
                    ky[:, si * N + mi * P: si * N + (mi + 1) * P],
                    kx[:, si * N:(si + 1) * N],
                    start=True, stop=True,
                )
                nc.scalar.activation(
                    expT[:, mi, :], etps[:], AF.Exp, accum_out=zy[:, mi:mi + 1],
                )

            rz = p_z.tile([P, 4], dt.float32, name="rz", tag="rz")
            nc.vector.reciprocal(rz[:], z[:])
            gr = p_z.tile([P, 4], dt.float32, name="gr", tag="gr")
            for bi in range(2):
                nc.vector.tensor_scalar(
                    gr[:, 2 * bi:2 * bi + 2], rz[:, 2 * bi:2 * bi + 2],
                    g_sb[bi][:, 0:1], 1.0 / SV,
                    op0=OP.mult, op1=OP.mult,
                )
            grg = gr

            samp_state[s] = (expE, expT, grg, res_tiles[s], vts)

        def stage_out(s):
            """DoubleRow output matmuls, scale+residual, store."""
            pr, si = s // 2, s % 2
            expE, expT, grg, res, vts = samp_state.pop(s)

            tmp = p_tmp.tile([P, 2, 2 * C], dt.bfloat16, name="tmp", tag="tmp")
            osb = p_out.tile([P, 2, 2 * C], dt.bfloat16, name="osb", tag="osb")
            late = s >= S - 2
            for bi in range(2):
                stat = expT if bi == 0 else expE
                vt = vts[(si, bi)]
                for qs in range(2):
                    ops = ps_o.tile([P, 2 * 512], dt.float32, name="pso", tag="pso")
                    sc = grg[:, 2 * bi + qs:2 * bi + qs + 1]
                    dst = tmp[:, bi, qs * C:(qs + 1) * C]
                    for h in range(2):
                        nc.tensor.matmul(
                            ops[:, h * 512:(h + 1) * 512],
                            stat[:, :, qs * P:(qs + 1) * P],
                            vt[:, :, h * 512:(h + 1) * 512],
                            start=True, stop=True, perf_mode=DR,
                        )
                        if late:
                            # last samples: drain each 512-half as soon as
                            # its matmul stops (ACT h0 / DVE h1 in parallel,
                            # different PSUM banks), then add+store the chunk
                            psh = ops[:, h * 512:(h + 1) * 512]
                            th = dst[:, h * 512:(h + 1) * 512]
                            if h == 0:
                                nc.scalar.activation(th, psh, AF.Identity, scale=sc)
                            else:
                                nc.vector.tensor_scalar(th, psh, sc, None, op0=OP.mult)
                    if late:
                        od = osb[:, bi, qs * C:(qs + 1) * C]
                        nc.vector.tensor_add(od, dst, res[:, bi, qs * C:(qs + 1) * C])
                        nc.sync.dma_start(out=oo[s][:, bi, qs * C:(qs + 1) * C], in_=od)
                        continue
                    if qs == 0:
                        nc.scalar.activation(dst, ops[:], AF.Identity, scale=sc)
                    else:
                        nc.vector.tensor_scalar(dst, ops[:], sc, None, op0=OP.mult)
                if not late:
                    eng = nc.gpsimd if (bi == 1 and s < 7) else nc.vector
                    eng.tensor_add(osb[:, bi, :], tmp[:, bi, :], res[:, bi, :])
            if not late:
                nc.sync.dma_start(out=oo[s], in_=osb[:])

        # software pipeline over sample pairs
        stage_proj(0)
        stage_proj(1)
        stage_attn(0)
        stage_attn(1)
        stage_out(0)
        stage_attn(2)
        stage_out(1)
        stage_proj(2)
        stage_attn(3)
        stage_out(2)
        stage_proj(3)
        stage_attn(4)
        stage_out(3)
        stage_attn(5)
        stage_out(4)
        stage_attn(6)
        stage_out(5)
        stage_attn(7)
        stage_out(6)
        stage_out(7)

    nc.finalize()
    return nc


def _ensure_axon_hooks_importable():
    try:
        import antenv.axon_hooks  # noqa: F401
    except Exception:
        import sys
        import types
        m = types.ModuleType("antenv.axon_hooks")
        m.get_axon_ntff_profile_hook = lambda: None
        m.set_axon_ntff_profile_hook = lambda h: None
        sys.modules["antenv.axon_hooks"] = m


def kernel(x, y, wk1, bk1, wk2, bk2, wv1, bv1, wv2, bv2, gamma1, gamma2):
    from concourse.bass_utils import run_bass_kernel_spmd

    _ensure_axon_hooks_importable()

    x = np.asarray(x, np.float32)
    y = np.asarray(y, np.float32)
    g1v = np.float32(np.asarray(gamma1).reshape(-1)[0])
    g2v = np.float32(np.asarray(gamma2).reshape(-1)[0])

    # activations: [B,C,H,W] -> [NCORES, NPAIR, P, T, 2N] fp8 (x16)
    def act_prep(a):
        r = a.reshape(NCORES, NPAIR, 2, T, P, N).transpose(0, 1, 4, 3, 2, 5)
        return np.ascontiguousarray(r).reshape(NCORES, NPAIR, P, T, 2 * N) * np.float32(SX)

    xq = act_prep(x).astype(_FP8)
    yq = act_prep(y).astype(_FP8)

    # residuals: xT + gamma*bv  -> [NCORES, S, P, 2C] bf16
    def res_prep(a, bv, gv):
        r = a.reshape(B, C, N).transpose(0, 2, 1) + (gv * np.asarray(bv, np.float32))[None, None, :]
        r = r.reshape(B, 2, P, C).transpose(0, 2, 1, 3)
        return np.ascontiguousarray(r).reshape(NCORES, S, P, 2 * C).astype(_BF16)

    rrq = np.stack([res_prep(x, bv1, g1v), res_prep(y, bv2, g2v)], axis=3)

    def wprep(w, cols):  # [cols, C] -> [P, T, cols] fp8 (x SW)
        r = np.asarray(w, np.float32).T.reshape(T, P, cols).transpose(1, 0, 2)
        return np.ascontiguousarray(r * np.float32(SW)).astype(_FP8)

    def wvprep(w):  # [C, C] -> [2, P, T, 512] fp8 (x SW), contiguous col-halves
        r = wprep(w, C).reshape(P, T, 2, 512).transpose(2, 0, 1, 3)
        return np.ascontiguousarray(r)

    common = {
        "wkt": np.stack([wprep(wk1, CK), wprep(wk2, CK)], axis=1),
        "wv1t": wvprep(wv1), "wv2t": wvprep(wv2),
        "cvec": np.ascontiguousarray(np.stack([
            np.asarray(bk1, np.float32).reshape(P),
            np.asarray(bk2, np.float32).reshape(P),
            np.full(P, g1v, np.float32),
            np.full(P, g2v, np.float32)], axis=1)),
    }

    nc = _build_program()
    in_maps = []
    for c in range(NCORES):
        in_maps.append({
            "xb": xq[c], "yb": yq[c], "rr": rrq[c],
            **common,
        })

    global LAST_RESULTS
    LAST_RESULTS = run_bass_kernel_spmd(nc, in_maps, list(range(NCORES)))
    res = LAST_RESULTS.results

    o = np.stack([res[c]["oo"] for c in range(NCORES)])  # [NC, S, P, 2, 2C]

    def unpack(r):  # [NC, S, P, 2C] bf16 -> [B, C, H, W] f32
        r = np.asarray(r, np.float32).reshape(B, P, 2, C).transpose(0, 3, 2, 1)
        return np.ascontiguousarray(r).reshape(B, C, 16, 16)

    return unpack(o[:, :, :, 0, :]), unpack(o[:, :, :, 1, :])

